# revision 53
# baseline (speedup 1.0000x reference)
"""Distributed multi-head attention kernel for 8 TRN2 NeuronCores.

Problem: x[2,2048,1024] -> qkv proj -> per-head RoPE (indexed by HEAD, a
fixed linear transform) -> attention (16 heads, d=64) -> out proj.

Sharding: core c handles batch c//4 and heads 4*(c%4) .. 4*(c%4)+3.
The out-projection partial sums are combined with a chunked ReduceScatter
over each 4-core group; the host only concatenates row blocks.

Host-side folds: RoPE rotation and the 1/sqrt(64) score scale are folded
into qkv_w columns; matmul inputs are pre-rounded to tf32 (float32r),
which streams at full PE rate. P = exp(S) and V are bf16 (same PE rate,
half the SBUF); softmax denominators come from a ones-column in V_aug.

Schedule: pass 1 projects K^T/V^T for the whole sequence (x^T streamed);
V^T is PE-transposed to V. Pass 2 projects Q^T one 512-wide chunk at a
time and immediately runs that chunk's attention: S^T matmuls -> exp
direct PSUM->SBUF on ACT -> P^T @ V_aug (accumulating softmax Z in row
64) -> reciprocal * broadcast -> out-projection -> per-chunk
ReduceScatter. Engines pipeline across chunks.
"""
import sys
for _p in ("/opt/trn_rl_repo", "/root/.axon_site/_ro/trn_rl_repo"):
    if _p not in sys.path:
        sys.path.insert(0, _p)

import numpy as np

from concourse import bacc, tile, bass_utils
from concourse import mybir

F32 = mybir.dt.float32
F32R = mybir.dt.float32r
BF16 = mybir.dt.bfloat16
EXP = mybir.ActivationFunctionType.Exp

HID = 1024
SEQ = 2048
HEADS = 16
D = 64
HPC = 4            # heads per core
N_CORES = 8
QC = 512           # q-chunk (free dim of scores matmuls)
NQ = SEQ // QC     # 4 q-chunks
KT = SEQ // 128    # 16 key tiles
VW = D + 1         # v_aug width per head (ones column at 64)


def _round_tf32(x):
    u = np.ascontiguousarray(x, dtype=np.float32).view(np.uint32).copy()
    u += 0xFFF + ((u >> 13) & 1)
    u &= np.uint32(0xFFFFE000)
    return u.view(np.float32)


def _rope_mats():
    """M_h [64,64] per head h: q_rot = q @ M_h (head-indexed RoPE quirk)."""
    j = np.arange(0, D, 2, dtype=np.float64) / D
    inv_freq = 1.0 / (10000.0 ** j)              # [32]
    h = np.arange(HEADS, dtype=np.float64)
    freqs = h[:, None] * inv_freq[None, :]       # [16, 32]
    cos = np.cos(freqs).astype(np.float32)
    sin = np.sin(freqs).astype(np.float32)
    mats = np.zeros((HEADS, D, D), np.float32)
    idx = np.arange(D // 2)
    for hh in range(HEADS):
        mats[hh, idx, idx] = cos[hh]
        mats[hh, D // 2 + idx, idx] = -sin[hh]
        mats[hh, idx, D // 2 + idx] = sin[hh]
        mats[hh, D // 2 + idx, D // 2 + idx] = cos[hh]
    return mats


_NC_CACHE = {}


def _build(with_collectives=True, n_cores=N_CORES):
    key = (with_collectives, n_cores)
    if key in _NC_CACHE:
        return _NC_CACHE[key]
    nc = bacc.Bacc("TRN2", target_bir_lowering=False, debug=False,
                   num_devices=n_cores)

    # weight column tiles ct: 0=q01 1=q23 2=k01 3=k23 4=v01 5=v23
    xt = nc.dram_tensor("xt", [HID, SEQ], F32R, kind="ExternalInput")
    wall = nc.dram_tensor("wall", [HID, 12 * D], F32R, kind="ExternalInput")
    w2 = nc.dram_tensor("w2", [HPC * D, HID], BF16, kind="ExternalInput")
    ball = nc.dram_tensor("ball", [128, 6], F32, kind="ExternalInput")
    bo = nc.dram_tensor("bo", [1, HID], F32R, kind="ExternalInput")
    ones_i = nc.dram_tensor("ones_i", [1, 128], F32R, kind="ExternalInput")
    ident = nc.dram_tensor("ident", [128, 128], BF16, kind="ExternalInput")
    vones = nc.dram_tensor("vones", [128, KT * HPC], BF16, kind="ExternalInput")
    if with_collectives:
        out_e = nc.dram_tensor("out", [QC, HID], F32, kind="ExternalOutput")
    else:
        out_e = nc.dram_tensor("out", [SEQ, HID], F32, kind="ExternalOutput")

    with tile.TileContext(nc) as tc:
        with tc.tile_pool(name="const", bufs=1) as cpool, \
             tc.tile_pool(name="work", bufs=1) as wpool, \
             tc.tile_pool(name="xts", bufs=1) as xpool, \
             tc.tile_pool(name="psum", bufs=1, space="PSUM") as pp, \
             tc.tile_pool(name="dram", bufs=1, space="DRAM") as dpool:

            # ---- constant loads
            wall_sb = cpool.tile([128, 8 * 768], F32R)     # k-tile k at [:, 768k:+768]
            w2_sb = cpool.tile([128, 2 * HID], BF16)
            ball_sb = cpool.tile([128, 6], F32)
            bo_sb = cpool.tile([1, HID], F32R)
            ones_sb = cpool.tile([1, 128], F32R)
            id_sb = cpool.tile([128, 128], BF16)
            nc.sync.dma_start(id_sb[:], ident.ap()[:])
            nc.sync.dma_start(ball_sb[:], ball.ap()[:])
            bob_sb = cpool.tile([128, HID], F32)

            # ---- persistent activations
            qkT_sb = wpool.tile([128, 4 * SEQ], BF16)   # col-tile ct at [:, ct*SEQ:+SEQ]
            vT_sb = wpool.tile([128, 2 * SEQ], BF16)
            v_sb = wpool.tile([128, KT * HPC * VW], BF16)
            outT_sb = wpool.tile([128, 2 * SEQ], BF16)

            def xt_dma(nq, k, eng):
                t = xpool.tile([128, 512], F32R, tag="xts", bufs=11,
                               name=f"xt_{nq}_{k}")
                eng.dma_start(t[:], xt.ap()[128 * k:128 * (k + 1),
                                            QC * nq:QC * (nq + 1)])
                return t

            CH0 = (0, 0, 512)
            # pt blocks filled during pass 1: q-chunk 0 heads 0,1 and head 2
            pt_00 = [wpool.tile([128, KT * QC], BF16, tag="pt", bufs=4,
                                name=f"pt00_{i}") for i in range(2)]
            pt_01 = [wpool.tile([128, KT * QC], BF16, tag="pt", bufs=4,
                                name=f"pt01_{i}") for i in range(2)]

            def scores_exp(ch, hp, kg, ptA, ptB):
                """S^T matmuls for head pair hp of one exp-group: G key tiles
                (G*q_len = 1024) of q-chunk ch; exp directly PSUM -> SBUF
                (bf16). ptA/ptB may be None to skip one half of the pair."""
                _, q_off, q_len = ch
                G = 1024 // q_len
                qslc = slice(SEQ * hp + q_off, SEQ * hp + q_off + q_len)
                halves = []
                if ptA is not None:
                    psA = pp.tile([128, 1024], F32, tag="s", bufs=2,
                                  name=f"psA_{q_off}_{hp}_{kg}")
                    halves.append((0, ptA, psA))
                if ptB is not None:
                    psB = pp.tile([128, 1024], F32, tag="s", bufs=2,
                                  name=f"psB_{q_off}_{hp}_{kg}")
                    halves.append((64, ptB, psB))
                for j in range(G):
                    kt = G * kg + j
                    kslc = slice(SEQ * (2 + hp) + 128 * kt,
                                 SEQ * (2 + hp) + 128 * (kt + 1))
                    for base, _pt, ps in halves:
                        nc.tensor.matmul(ps[:, q_len * j:q_len * (j + 1)],
                                         lhsT=qkT_sb[base:base + 64, kslc],
                                         rhs=qkT_sb[base:base + 64, qslc],
                                         start=True, stop=True,
                                         tile_position=(base, 0))
                for base, _pt, ps in halves:
                    nc.scalar.activation(_pt[:, 1024 * kg:1024 * (kg + 1)],
                                         ps[:], EXP)

            def scores_exp1(ch, hp, half, kg, pt):
                scores_exp(ch, hp, kg, *((pt, None) if half == 0 else (None, pt)))

            def v_mm(oacc, h, kt, pt, q_len, start, stop):
                nc.tensor.matmul(
                    oacc[:, :q_len],
                    lhsT=v_sb[:, VW * HPC * kt + VW * h:VW * HPC * kt + VW * (h + 1)],
                    rhs=pt[:, q_len * kt:q_len * (kt + 1)],
                    start=start, stop=stop)

            def normalize(ch, hp, half, oacc):
                _, q_off, q_len = ch
                h = 2 * hp + half
                rz = wpool.tile([1, 512], F32R, tag="rz", bufs=2,
                                name=f"rz_{q_off}_{h}")
                with nc.allow_low_precision(reason="tf32 recip feeds bcast matmul"):
                    nc.vector.reciprocal(rz[:, :q_len], oacc[D:D + 1, :q_len])
                bcm = pp.tile([64, 512], F32, tag="pr", bufs=2, name=f"bcm_{q_off}_{h}")
                nc.tensor.matmul(bcm[:, :q_len], lhsT=ones_sb[:, :64],
                                 rhs=rz[:, :q_len], start=True, stop=True)
                bc = wpool.tile([64, 512], F32, tag="bc", bufs=2, name=f"bc_{q_off}_{h}")
                nc.vector.tensor_copy(bc[:, :q_len], bcm[:, :q_len])
                nc.vector.tensor_tensor(
                    outT_sb[64 * half:64 * (half + 1),
                            SEQ * hp + q_off:SEQ * hp + q_off + q_len],
                    oacc[0:D, :q_len], bc[:, :q_len],
                    mybir.AluOpType.mult)

            # ---- pass 1: project K^T, then Q^T, then V^T chunk by chunk
            # (3 sweeps over resident x^T tiles); early scores+exp for
            # q-chunk 0 keep ACT busy while the PE projects.
            for nq in range(NQ):
                sQ = pp.tile([128, 1024], F32, tag="s", bufs=2, name=f"sQ_{nq}")
                sK = pp.tile([128, 1024], F32, tag="s", bufs=2, name=f"sK_{nq}")
                vA = pp.tile([128, 512], F32, tag="oacc", bufs=2, name=f"vA_{nq}")
                vB = pp.tile([128, 512], F32, tag="pr", bufs=2, name=f"vB_{nq}")
                xts = []
                for k in range(8):
                    if nq == 0:
                        nc.gpsimd.dma_start(wall_sb[:, 768 * k:768 * (k + 1)],
                                            wall.ap()[128 * k:128 * (k + 1), :])
                    xt_t = xt_dma(nq, k, nc.sync if k % 2 == 0 else nc.scalar)
                    xts.append(xt_t)
                    for j, ct in enumerate((2, 3)):
                        nc.tensor.matmul(
                            sK[:, 512 * j:512 * (j + 1)],
                            lhsT=wall_sb[:, 768 * k + 128 * ct:768 * k + 128 * (ct + 1)],
                            rhs=xt_t[:], start=(k == 0), stop=(k == 7))
                    if nq == 0:
                        # chunk 0: Q interleaved so the first scores fire asap
                        for j, ct in enumerate((0, 1)):
                            nc.tensor.matmul(
                                sQ[:, 512 * j:512 * (j + 1)],
                                lhsT=wall_sb[:, 768 * k + 128 * ct:768 * k + 128 * (ct + 1)],
                                rhs=xt_t[:], start=(k == 0), stop=(k == 7))
                for j, ct in enumerate((2, 3)):
                    nc.vector.tensor_scalar_add(
                        qkT_sb[:, SEQ * ct + QC * nq:SEQ * ct + QC * (nq + 1)],
                        sK[:, 512 * j:512 * (j + 1)], ball_sb[:, ct:ct + 1])
                if nq > 0:
                    scores_exp(CH0, 0, 2 * nq, pt_00[0], pt_00[1])
                else:
                    for j, ct in enumerate((0, 1)):
                        nc.vector.tensor_scalar_add(
                            qkT_sb[:, SEQ * ct + QC * nq:SEQ * ct + QC * (nq + 1)],
                            sQ[:, 512 * j:512 * (j + 1)], ball_sb[:, ct:ct + 1])
                    scores_exp(CH0, 0, 0, pt_00[0], pt_00[1])
                if nq > 0:
                    for k in range(8):
                        for j, ct in enumerate((0, 1)):
                            nc.tensor.matmul(
                                sQ[:, 512 * j:512 * (j + 1)],
                                lhsT=wall_sb[:, 768 * k + 128 * ct:768 * k + 128 * (ct + 1)],
                                rhs=xts[k][:], start=(k == 0), stop=(k == 7))
                    for j, ct in enumerate((0, 1)):
                        nc.vector.tensor_scalar_add(
                            qkT_sb[:, SEQ * ct + QC * nq:SEQ * ct + QC * (nq + 1)],
                            sQ[:, 512 * j:512 * (j + 1)], ball_sb[:, ct:ct + 1])
                if nq == 0:
                    pass
                scores_exp1(CH0, 1, 0, 2 * nq, pt_01[0])
                scores_exp(CH0, 0, 2 * nq + 1, pt_00[0], pt_00[1])
                for k in range(8):
                    nc.tensor.matmul(
                        vA[:], lhsT=wall_sb[:, 768 * k + 512:768 * k + 640],
                        rhs=xts[k][:], start=(k == 0), stop=(k == 7))
                    nc.tensor.matmul(
                        vB[:], lhsT=wall_sb[:, 768 * k + 640:768 * k + 768],
                        rhs=xts[k][:], start=(k == 0), stop=(k == 7))
                nc.vector.tensor_scalar_add(
                    vT_sb[:, QC * nq:QC * (nq + 1)], vA[:], ball_sb[:, 4:5])
                nc.vector.tensor_scalar_add(
                    vT_sb[:, SEQ + QC * nq:SEQ + QC * (nq + 1)], vB[:], ball_sb[:, 5:6])
                # V^T -> V (natural, bf16) for this quarter of the keys
                for cv in range(2):
                    for st in range(4 * nq, 4 * nq + 4):
                        tp = pp.tile([128, 128], BF16, tag="pr", bufs=2,
                                     name=f"tp_{cv}_{st}")
                        nc.tensor.transpose(
                            tp[:], vT_sb[:, SEQ * cv + 128 * st:SEQ * cv + 128 * (st + 1)],
                            id_sb[:])
                        dst = v_sb[:, VW * HPC * st + 2 * VW * cv:VW * HPC * st + 2 * VW * (cv + 1)]
                        nc.vector.tensor_copy(
                            dst.rearrange("p (h w) -> p h w", h=2, w=VW)[:, :, :D],
                            tp[:].rearrange("p (h w) -> p h w", h=2, w=D),
                        )
                scores_exp1(CH0, 1, 0, 2 * nq + 1, pt_01[0])

            # ones columns of v_aug: one strided DMA ([128, 64] with free stride VW)
            nc.sync.dma_start(
                v_sb[:].rearrange("p (i w) -> p i w", i=KT * HPC, w=VW)[:, :, D],
                vones.ap()[:],
            )
            # ---- deferred constants (needed from pass 2 on)
            for k in range(2):
                nc.sync.dma_start(w2_sb[:, HID * k:HID * (k + 1)],
                                  w2.ap()[128 * k:128 * (k + 1), :])
            nc.sync.dma_start(bo_sb[:], bo.ap()[:])
            nc.sync.dma_start(ones_sb[:], ones_i.ap()[:])
            # out-bias broadcast ([1,N] -> [128,N] via K=1 ones matmul)
            for nn in range(2):
                ps_bo = pp.tile([128, 512], F32, tag="pr", bufs=2)
                nc.tensor.matmul(ps_bo[:], lhsT=ones_sb[:, :128],
                                 rhs=bo_sb[:, 512 * nn:512 * (nn + 1)],
                                 start=True, stop=True)
                nc.vector.tensor_copy(bob_sb[:, 512 * nn:512 * (nn + 1)], ps_bo[:])

            def out_proj(nq):
                rs_in = dpool.tile([QC, HID], F32, tag="rsin", bufs=2, name=f"rsin_{nq}")
                for qt in range(4):
                    ob = wpool.tile([128, HID], F32, tag="ob", bufs=3, name=f"ob_{nq}_{qt}")
                    for nn in range(2):
                        pso = pp.tile([128, 512], F32, tag="pr", bufs=2,
                                      name=f"pso_{nq}_{qt}_{nn}")
                        for kk in range(2):
                            nc.tensor.matmul(
                                pso[:],
                                lhsT=outT_sb[:, SEQ * kk + QC * nq + 128 * qt:SEQ * kk + QC * nq + 128 * (qt + 1)],
                                rhs=w2_sb[:, HID * kk + 512 * nn:HID * kk + 512 * (nn + 1)],
                                start=(kk == 0), stop=(kk == 1))
                        nc.vector.tensor_tensor(
                            ob[:, 512 * nn:512 * (nn + 1)], pso[:],
                            bob_sb[:, 512 * nn:512 * (nn + 1)],
                            mybir.AluOpType.add)
                    if with_collectives:
                        nc.sync.dma_start(rs_in[128 * qt:128 * (qt + 1), :], ob[:, :HID])
                        if nq == NQ - 1 and qt == 1:
                            do_rs(rs_in[0:256, :], 64, f"{nq}a", 128 * nq)
                    else:
                        nc.sync.dma_start(
                            out_e.ap()[QC * nq + 128 * qt:QC * nq + 128 * (qt + 1), :],
                            ob[:, :HID])
                if with_collectives:
                    if nq == NQ - 1:
                        do_rs(rs_in[256:512, :], 64, f"{nq}b", 128 * nq + 64)
                    else:
                        do_rs(rs_in[:], 128, str(nq), 128 * nq)

            def do_rs(src_ap, rows, label, out_row):
                rs_out = dpool.tile([rows, HID], F32, tag="rsout", bufs=2,
                                    name=f"rsout_{label}")
                nc.gpsimd.collective_compute(
                    "ReduceScatter",
                    mybir.AluOpType.add,
                    replica_groups=[[0, 1, 2, 3], [4, 5, 6, 7]][:max(1, n_cores // 4)],
                    ins=[src_ap.opt()],
                    outs=[rs_out[:].opt()],
                )
                nc.sync.dma_start(out_e.ap()[out_row:out_row + rows, :], rs_out[:])

            # q-chunks: (index, q_off, q_len); the final 512 rows are two
            # 256-row chunks so the drain tail and last RS are half-sized.
            CHUNKS = [(0, 0, 512), (1, 512, 512), (2, 1024, 512),
                      (3, 1536, 256), (4, 1792, 256)]
            # per-chunk output row offset in out_e (rank-relative)
            OUT_ROW = {0: 0, 1: 128, 2: 256, 3: 384, 4: 448}

            # ---- pass 2: per q-chunk attention; out proj of the previous
            # chunk is emitted inside the next chunk's score loop so the PE
            # keeps feeding ACT at chunk boundaries.
            def out_proj(ch):
                idx, q_off, q_len = ch
                nqt = q_len // 128
                rs_in = dpool.tile([QC, HID], F32, tag="rsin", bufs=2,
                                   name=f"rsin_{idx}")
                for qt in range(nqt):
                    ob = wpool.tile([128, HID], F32, tag="ob", bufs=3,
                                    name=f"ob_{idx}_{qt}")
                    for nn in range(2):
                        pso = pp.tile([128, 512], F32, tag="pr", bufs=2,
                                      name=f"pso_{idx}_{qt}_{nn}")
                        for kk in range(2):
                            nc.tensor.matmul(
                                pso[:],
                                lhsT=outT_sb[:, SEQ * kk + q_off + 128 * qt:SEQ * kk + q_off + 128 * (qt + 1)],
                                rhs=w2_sb[:, HID * kk + 512 * nn:HID * kk + 512 * (nn + 1)],
                                start=(kk == 0), stop=(kk == 1))
                        nc.vector.tensor_tensor(
                            ob[:, 512 * nn:512 * (nn + 1)], pso[:],
                            bob_sb[:, 512 * nn:512 * (nn + 1)],
                            mybir.AluOpType.add)
                    if with_collectives:
                        nc.sync.dma_start(rs_in[128 * qt:128 * (qt + 1), :], ob[:, :HID])
                    else:
                        nc.sync.dma_start(
                            out_e.ap()[q_off + 128 * qt:q_off + 128 * (qt + 1), :],
                            ob[:, :HID])
                if with_collectives:
                    do_rs(rs_in[0:q_len, :], q_len // 4, str(idx), OUT_ROW[idx])

            pending = None
            pending_norm = []
            for ch in CHUNKS:
                idx, q_off, q_len = ch
                G = 1024 // q_len
                for hp in range(2):
                    early_A, early_B = (pt_00 if (idx == 0 and hp == 0) else
                                        (pt_01[0], None) if (idx == 0 and hp == 1) else
                                        (None, None))
                    ptA = early_A or wpool.tile([128, KT * q_len], BF16, tag="pt",
                                                bufs=5, name=f"ptA_{idx}_{hp}")
                    ptB = early_B or (pt_01[1] if (idx == 0 and hp == 1) else
                                      wpool.tile([128, KT * q_len], BF16, tag="pt",
                                                 bufs=5, name=f"ptB_{idx}_{hp}"))
                    oaccs = [pp.tile([VW, 512], F32, tag="oacc", bufs=2,
                                     name=f"oacc_{idx}_{2 * hp + half}")
                             for half in range(2)]
                    def v_group(kg):
                        for half, pt in ((0, ptA), (1, ptB)):
                            for j in range(G):
                                kt = G * kg + j
                                v_mm(oaccs[half], 2 * hp + half, kt, pt, q_len,
                                     kt == 0, kt == KT - 1)
                    # V matmuls trail the scores/exp by one group so the PE
                    # never head-of-line blocks on the exp it just requested
                    for kg in range(KT // G):
                        scores_exp(ch, hp, kg,
                                   None if early_A is not None else ptA,
                                   None if early_B is not None else ptB)
                        if kg == 0:
                            for args in pending_norm:
                                normalize(*args)
                            pending_norm = []
                        else:
                            v_group(kg - 1)
                        if kg == 1 and hp == 0 and pending is not None:
                            out_proj(pending)
                            pending = None
                    v_group(KT // G - 1)
                    for half in range(2):
                        pending_norm.append((ch, hp, half, oaccs[half]))
                pending = ch
            for args in pending_norm:
                normalize(*args)
            out_proj(pending)

    nc.compile()
    _NC_CACHE[key] = nc
    return nc


def _prep_in_maps(x, qkv_w, qkv_b, out_w, out_b):
    mats = _rope_mats()
    x = np.asarray(x, np.float32)
    qkv_w = np.asarray(qkv_w, np.float32)
    qkv_b = np.asarray(qkv_b, np.float32)
    out_w = np.asarray(out_w, np.float32)
    out_b = np.asarray(out_b, np.float32)

    # per-head slices of interleaved qkv (head h owns cols 192h .. 192h+192)
    wq = np.stack([qkv_w[:, 192 * h:192 * h + 64] for h in range(HEADS)])      # [16,1024,64]
    wk = np.stack([qkv_w[:, 192 * h + 64:192 * h + 128] for h in range(HEADS)])
    wv = np.stack([qkv_w[:, 192 * h + 128:192 * h + 192] for h in range(HEADS)])
    bq = np.stack([qkv_b[192 * h:192 * h + 64] for h in range(HEADS)])
    bk = np.stack([qkv_b[192 * h + 64:192 * h + 128] for h in range(HEADS)])
    bvv = np.stack([qkv_b[192 * h + 128:192 * h + 192] for h in range(HEADS)])

    import ml_dtypes
    scale = 1.0 / np.sqrt(D)
    wq_r = np.einsum("hij,hjk->hik", wq, mats) * scale
    bq_r = np.einsum("hj,hjk->hk", bq, mats) * scale
    wk_r = np.einsum("hij,hjk->hik", wk, mats)
    bk_r = np.einsum("hj,hjk->hk", bk, mats)

    in_maps = []
    for c in range(N_CORES):
        g, r = divmod(c, 4)
        hs = [4 * r + i for i in range(HPC)]
        xt = _round_tf32(x[g].T)                                            # [1024, 2048]
        wall_c = np.concatenate([wq_r[h] for h in hs] + [wk_r[h] for h in hs]
                                + [wv[h] for h in hs], axis=1)              # [1024, 768]
        w2_c = out_w[256 * r:256 * (r + 1), :]                              # [256, 1024]
        ball_c = np.concatenate([bq_r[h] for h in hs] + [bk_r[h] for h in hs]
                                + [bvv[h] for h in hs])                     # [768]
        bo_c = (out_b[None, :] if r == 0 else np.zeros((1, HID), np.float32))
        in_maps.append({
            "xt": xt,
            "wall": _round_tf32(wall_c),
            "w2": w2_c.astype(ml_dtypes.bfloat16),
            "ball": ball_c.reshape(6, 128).T.copy().astype(np.float32),
            "bo": _round_tf32(bo_c),
            "ones_i": np.ones((1, 128), np.float32),
            "ident": np.eye(128, dtype=ml_dtypes.bfloat16),
            "vones": np.ones((128, KT * HPC), ml_dtypes.bfloat16),
        })
    return in_maps


def kernel(x, qkv_w, qkv_b, out_w, out_b):
    in_maps = _prep_in_maps(x, qkv_w, qkv_b, out_w, out_b)
    nc = _build(with_collectives=True)
    res = None
    for attempt, backoff in enumerate((10, 20, 40, 60, 0)):
        try:
            res = bass_utils.run_bass_kernel_spmd(nc, in_maps,
                                                  core_ids=list(range(N_CORES)))
            break
        except Exception:
            if backoff == 0:
                raise
            import time as _time
            _time.sleep(backoff)
    out = np.empty((2, SEQ, HID), np.float32)
    for c in range(N_CORES):
        g, r = divmod(c, 4)
        o = res.results[c]["out"]            # [512, 1024]
        for j in range(3):
            out[g, 512 * j + 128 * r:512 * j + 128 * r + 128] = o[128 * j:128 * (j + 1)]
        # chunk 3 was reduced as two half-sized ReduceScatters
        for s in range(2):
            out[g, 1536 + 256 * s + 64 * r:1536 + 256 * s + 64 * (r + 1)] = \
                o[384 + 64 * s:384 + 64 * (s + 1)]
    return out


# revision 61
# speedup vs baseline: 1.0072x; 1.0072x over previous
"""Distributed multi-head attention kernel for 8 TRN2 NeuronCores.

Problem: x[2,2048,1024] -> qkv proj -> per-head RoPE (indexed by HEAD, a
fixed linear transform) -> attention (16 heads, d=64) -> out proj.

Sharding: core c handles batch c//4 and heads 4*(c%4) .. 4*(c%4)+3.
The out-projection partial sums are combined with a chunked ReduceScatter
over each 4-core group; the host only concatenates row blocks.

Host-side folds: RoPE rotation and the 1/sqrt(64) score scale are folded
into qkv_w columns; matmul inputs are pre-rounded to tf32 (float32r),
which streams at full PE rate. P = exp(S) and V are bf16 (same PE rate,
half the SBUF); softmax denominators come from a ones-column in V_aug.

Schedule: pass 1 projects K^T/V^T for the whole sequence (x^T streamed);
V^T is PE-transposed to V. Pass 2 projects Q^T one 512-wide chunk at a
time and immediately runs that chunk's attention: S^T matmuls -> exp
direct PSUM->SBUF on ACT -> P^T @ V_aug (accumulating softmax Z in row
64) -> reciprocal * broadcast -> out-projection -> per-chunk
ReduceScatter. Engines pipeline across chunks.
"""
import sys
for _p in ("/opt/trn_rl_repo", "/root/.axon_site/_ro/trn_rl_repo"):
    if _p not in sys.path:
        sys.path.insert(0, _p)

import numpy as np

from concourse import bacc, tile, bass_utils
from concourse import mybir

F32 = mybir.dt.float32
F32R = mybir.dt.float32r
BF16 = mybir.dt.bfloat16
EXP = mybir.ActivationFunctionType.Exp

HID = 1024
SEQ = 2048
HEADS = 16
D = 64
HPC = 4            # heads per core
N_CORES = 8
QC = 512           # q-chunk (free dim of scores matmuls)
NQ = SEQ // QC     # 4 q-chunks
KT = SEQ // 128    # 16 key tiles
VW = D + 1         # v_aug width per head (ones column at 64)


def _round_tf32(x):
    u = np.ascontiguousarray(x, dtype=np.float32).view(np.uint32).copy()
    u += 0xFFF + ((u >> 13) & 1)
    u &= np.uint32(0xFFFFE000)
    return u.view(np.float32)


def _rope_mats():
    """M_h [64,64] per head h: q_rot = q @ M_h (head-indexed RoPE quirk)."""
    j = np.arange(0, D, 2, dtype=np.float64) / D
    inv_freq = 1.0 / (10000.0 ** j)              # [32]
    h = np.arange(HEADS, dtype=np.float64)
    freqs = h[:, None] * inv_freq[None, :]       # [16, 32]
    cos = np.cos(freqs).astype(np.float32)
    sin = np.sin(freqs).astype(np.float32)
    mats = np.zeros((HEADS, D, D), np.float32)
    idx = np.arange(D // 2)
    for hh in range(HEADS):
        mats[hh, idx, idx] = cos[hh]
        mats[hh, D // 2 + idx, idx] = -sin[hh]
        mats[hh, idx, D // 2 + idx] = sin[hh]
        mats[hh, D // 2 + idx, D // 2 + idx] = cos[hh]
    return mats


_NC_CACHE = {}


def _build(with_collectives=True, n_cores=N_CORES):
    key = (with_collectives, n_cores)
    if key in _NC_CACHE:
        return _NC_CACHE[key]
    nc = bacc.Bacc("TRN2", target_bir_lowering=False, debug=False,
                   num_devices=n_cores)

    # weight column tiles ct: 0=q01 1=q23 2=k01 3=k23 4=v01 5=v23
    xt = nc.dram_tensor("xt", [HID, SEQ], F32R, kind="ExternalInput")
    wall = nc.dram_tensor("wall", [HID, 12 * D], F32R, kind="ExternalInput")
    w2 = nc.dram_tensor("w2", [HPC * D, HID], BF16, kind="ExternalInput")
    ball = nc.dram_tensor("ball", [128, 6], F32, kind="ExternalInput")
    bo = nc.dram_tensor("bo", [1, HID], F32R, kind="ExternalInput")
    ones_i = nc.dram_tensor("ones_i", [1, 128], F32R, kind="ExternalInput")
    ident = nc.dram_tensor("ident", [128, 128], BF16, kind="ExternalInput")
    vones = nc.dram_tensor("vones", [128, KT * HPC], BF16, kind="ExternalInput")
    if with_collectives:
        out_e = nc.dram_tensor("out", [QC, HID], F32, kind="ExternalOutput")
    else:
        out_e = nc.dram_tensor("out", [SEQ, HID], F32, kind="ExternalOutput")

    with tile.TileContext(nc) as tc:
        with tc.tile_pool(name="const", bufs=1) as cpool, \
             tc.tile_pool(name="work", bufs=1) as wpool, \
             tc.tile_pool(name="xts", bufs=1) as xpool, \
             tc.tile_pool(name="psum", bufs=1, space="PSUM") as pp, \
             tc.tile_pool(name="dram", bufs=1, space="DRAM") as dpool:

            # ---- constant loads
            wall_sb = cpool.tile([128, 8 * 768], F32R)     # k-tile k at [:, 768k:+768]
            w2_sb = cpool.tile([128, 2 * HID], BF16)
            ball_sb = cpool.tile([128, 6], F32)
            bo_sb = cpool.tile([1, HID], F32R)
            ones_sb = cpool.tile([1, 128], F32R)
            id_sb = cpool.tile([128, 128], BF16)
            nc.sync.dma_start(id_sb[:], ident.ap()[:])
            nc.sync.dma_start(ball_sb[:], ball.ap()[:])
            bob_sb = cpool.tile([128, HID], F32)

            # ---- persistent activations
            qkT_sb = wpool.tile([128, 4 * SEQ], BF16)   # col-tile ct at [:, ct*SEQ:+SEQ]
            vT_sb = wpool.tile([128, 2 * SEQ], BF16)
            v_sb = wpool.tile([128, KT * HPC * VW], BF16)
            outT_sb = wpool.tile([128, 2 * SEQ], BF16)

            def xt_dma(nq, k, eng):
                t = xpool.tile([128, 512], F32R, tag="xts", bufs=11,
                               name=f"xt_{nq}_{k}")
                eng.dma_start(t[:], xt.ap()[128 * k:128 * (k + 1),
                                            QC * nq:QC * (nq + 1)])
                return t

            CH0 = (0, 0, 512)
            # pt blocks filled during pass 1: q-chunk 0 heads 0,1 and head 2
            pt_00 = [wpool.tile([128, KT * QC], BF16, tag="pt", bufs=4,
                                name=f"pt00_{i}") for i in range(2)]
            pt_01 = [wpool.tile([128, KT * QC], BF16, tag="pt", bufs=4,
                                name=f"pt01_{i}") for i in range(2)]

            def scores_exp(ch, hp, kg, ptA, ptB):
                """S^T matmuls for head pair hp of one exp-group: G key tiles
                (G*q_len = 1024) of q-chunk ch; exp directly PSUM -> SBUF
                (bf16). ptA/ptB may be None to skip one half of the pair."""
                _, q_off, q_len = ch
                G = 1024 // q_len
                qslc = slice(SEQ * hp + q_off, SEQ * hp + q_off + q_len)
                halves = []
                if ptA is not None:
                    psA = pp.tile([128, 1024], F32, tag="s", bufs=2,
                                  name=f"psA_{q_off}_{hp}_{kg}")
                    halves.append((0, ptA, psA))
                if ptB is not None:
                    psB = pp.tile([128, 1024], F32, tag="s", bufs=2,
                                  name=f"psB_{q_off}_{hp}_{kg}")
                    halves.append((64, ptB, psB))
                for j in range(G):
                    kt = G * kg + j
                    kslc = slice(SEQ * (2 + hp) + 128 * kt,
                                 SEQ * (2 + hp) + 128 * (kt + 1))
                    for base, _pt, ps in halves:
                        nc.tensor.matmul(ps[:, q_len * j:q_len * (j + 1)],
                                         lhsT=qkT_sb[base:base + 64, kslc],
                                         rhs=qkT_sb[base:base + 64, qslc],
                                         start=True, stop=True,
                                         tile_position=(base, 0))
                for base, _pt, ps in halves:
                    nc.scalar.activation(_pt[:, 1024 * kg:1024 * (kg + 1)],
                                         ps[:], EXP)

            def scores_exp1(ch, hp, half, kg, pt):
                scores_exp(ch, hp, kg, *((pt, None) if half == 0 else (None, pt)))

            def v_mm(oacc, h, kt, pt, q_len, start, stop):
                nc.tensor.matmul(
                    oacc[:, :q_len],
                    lhsT=v_sb[:, VW * HPC * kt + VW * h:VW * HPC * kt + VW * (h + 1)],
                    rhs=pt[:, q_len * kt:q_len * (kt + 1)],
                    start=start, stop=stop)

            def normalize(ch, hp, half, oacc):
                _, q_off, q_len = ch
                h = 2 * hp + half
                rz = wpool.tile([1, 512], F32R, tag="rz", bufs=2,
                                name=f"rz_{q_off}_{h}")
                with nc.allow_low_precision(reason="tf32 recip feeds bcast matmul"):
                    nc.vector.reciprocal(rz[:, :q_len], oacc[D:D + 1, :q_len])
                # copy the unnormalized head output out of PSUM while the
                # reciprocal/broadcast run; the multiply then reads the
                # broadcast straight from PSUM (one DVE hop shorter)
                ou = wpool.tile([64, 512], F32, tag="bc", bufs=2, name=f"ou_{q_off}_{h}")
                nc.vector.tensor_copy(ou[:, :q_len], oacc[0:D, :q_len])
                bcm = pp.tile([64, 512], F32, tag="pr", bufs=2, name=f"bcm_{q_off}_{h}")
                nc.tensor.matmul(bcm[:, :q_len], lhsT=ones_sb[:, :64],
                                 rhs=rz[:, :q_len], start=True, stop=True)
                nc.vector.tensor_tensor(
                    outT_sb[64 * half:64 * (half + 1),
                            SEQ * hp + q_off:SEQ * hp + q_off + q_len],
                    bcm[:, :q_len], ou[:, :q_len],
                    mybir.AluOpType.mult)

            # ---- pass 1: project K^T, then Q^T, then V^T chunk by chunk
            # (3 sweeps over resident x^T tiles); early scores+exp for
            # q-chunk 0 keep ACT busy while the PE projects.
            for nq in range(NQ):
                sQ = pp.tile([128, 1024], F32, tag="s", bufs=2, name=f"sQ_{nq}")
                sK = pp.tile([128, 1024], F32, tag="s", bufs=2, name=f"sK_{nq}")
                vA = pp.tile([128, 512], F32, tag="oacc", bufs=2, name=f"vA_{nq}")
                vB = pp.tile([128, 512], F32, tag="pr", bufs=2, name=f"vB_{nq}")
                xts = []
                for k in range(8):
                    if nq == 0:
                        nc.gpsimd.dma_start(wall_sb[:, 768 * k:768 * (k + 1)],
                                            wall.ap()[128 * k:128 * (k + 1), :])
                    xt_t = xt_dma(nq, k, nc.sync if k % 2 == 0 else nc.scalar)
                    xts.append(xt_t)
                    for j, ct in enumerate((2, 3)):
                        nc.tensor.matmul(
                            sK[:, 512 * j:512 * (j + 1)],
                            lhsT=wall_sb[:, 768 * k + 128 * ct:768 * k + 128 * (ct + 1)],
                            rhs=xt_t[:], start=(k == 0), stop=(k == 7))
                    if nq == 0:
                        # chunk 0: Q interleaved so the first scores fire asap
                        for j, ct in enumerate((0, 1)):
                            nc.tensor.matmul(
                                sQ[:, 512 * j:512 * (j + 1)],
                                lhsT=wall_sb[:, 768 * k + 128 * ct:768 * k + 128 * (ct + 1)],
                                rhs=xt_t[:], start=(k == 0), stop=(k == 7))
                for j, ct in enumerate((2, 3)):
                    nc.vector.tensor_scalar_add(
                        qkT_sb[:, SEQ * ct + QC * nq:SEQ * ct + QC * (nq + 1)],
                        sK[:, 512 * j:512 * (j + 1)], ball_sb[:, ct:ct + 1])
                if nq > 0:
                    scores_exp(CH0, 0, 2 * nq, pt_00[0], pt_00[1])
                else:
                    for j, ct in enumerate((0, 1)):
                        nc.vector.tensor_scalar_add(
                            qkT_sb[:, SEQ * ct + QC * nq:SEQ * ct + QC * (nq + 1)],
                            sQ[:, 512 * j:512 * (j + 1)], ball_sb[:, ct:ct + 1])
                    scores_exp(CH0, 0, 0, pt_00[0], pt_00[1])
                if nq > 0:
                    for k in range(8):
                        for j, ct in enumerate((0, 1)):
                            nc.tensor.matmul(
                                sQ[:, 512 * j:512 * (j + 1)],
                                lhsT=wall_sb[:, 768 * k + 128 * ct:768 * k + 128 * (ct + 1)],
                                rhs=xts[k][:], start=(k == 0), stop=(k == 7))
                    for j, ct in enumerate((0, 1)):
                        nc.vector.tensor_scalar_add(
                            qkT_sb[:, SEQ * ct + QC * nq:SEQ * ct + QC * (nq + 1)],
                            sQ[:, 512 * j:512 * (j + 1)], ball_sb[:, ct:ct + 1])
                if nq == 0:
                    pass
                scores_exp1(CH0, 1, 0, 2 * nq, pt_01[0])
                scores_exp(CH0, 0, 2 * nq + 1, pt_00[0], pt_00[1])
                for k in range(8):
                    nc.tensor.matmul(
                        vA[:], lhsT=wall_sb[:, 768 * k + 512:768 * k + 640],
                        rhs=xts[k][:], start=(k == 0), stop=(k == 7))
                    nc.tensor.matmul(
                        vB[:], lhsT=wall_sb[:, 768 * k + 640:768 * k + 768],
                        rhs=xts[k][:], start=(k == 0), stop=(k == 7))
                nc.vector.tensor_scalar_add(
                    vT_sb[:, QC * nq:QC * (nq + 1)], vA[:], ball_sb[:, 4:5])
                nc.vector.tensor_scalar_add(
                    vT_sb[:, SEQ + QC * nq:SEQ + QC * (nq + 1)], vB[:], ball_sb[:, 5:6])
                # V^T -> V (natural, bf16) for this quarter of the keys
                for cv in range(2):
                    for st in range(4 * nq, 4 * nq + 4):
                        tp = pp.tile([128, 128], BF16, tag="pr", bufs=2,
                                     name=f"tp_{cv}_{st}")
                        nc.tensor.transpose(
                            tp[:], vT_sb[:, SEQ * cv + 128 * st:SEQ * cv + 128 * (st + 1)],
                            id_sb[:])
                        dst = v_sb[:, VW * HPC * st + 2 * VW * cv:VW * HPC * st + 2 * VW * (cv + 1)]
                        nc.vector.tensor_copy(
                            dst.rearrange("p (h w) -> p h w", h=2, w=VW)[:, :, :D],
                            tp[:].rearrange("p (h w) -> p h w", h=2, w=D),
                        )
                scores_exp1(CH0, 1, 0, 2 * nq + 1, pt_01[0])

            # ones columns of v_aug: one strided DMA ([128, 64] with free stride VW)
            nc.sync.dma_start(
                v_sb[:].rearrange("p (i w) -> p i w", i=KT * HPC, w=VW)[:, :, D],
                vones.ap()[:],
            )
            # ---- deferred constants (needed from pass 2 on)
            for k in range(2):
                nc.sync.dma_start(w2_sb[:, HID * k:HID * (k + 1)],
                                  w2.ap()[128 * k:128 * (k + 1), :])
            nc.sync.dma_start(bo_sb[:], bo.ap()[:])
            nc.sync.dma_start(ones_sb[:], ones_i.ap()[:])
            # out-bias broadcast ([1,N] -> [128,N] via K=1 ones matmul)
            for nn in range(2):
                ps_bo = pp.tile([128, 512], F32, tag="pr", bufs=2)
                nc.tensor.matmul(ps_bo[:], lhsT=ones_sb[:, :128],
                                 rhs=bo_sb[:, 512 * nn:512 * (nn + 1)],
                                 start=True, stop=True)
                nc.vector.tensor_copy(bob_sb[:, 512 * nn:512 * (nn + 1)], ps_bo[:])

            def out_proj(nq):
                rs_in = dpool.tile([QC, HID], F32, tag="rsin", bufs=2, name=f"rsin_{nq}")
                for qt in range(4):
                    ob = wpool.tile([128, HID], F32, tag="ob", bufs=3, name=f"ob_{nq}_{qt}")
                    for nn in range(2):
                        pso = pp.tile([128, 512], F32, tag="pr", bufs=2,
                                      name=f"pso_{nq}_{qt}_{nn}")
                        for kk in range(2):
                            nc.tensor.matmul(
                                pso[:],
                                lhsT=outT_sb[:, SEQ * kk + QC * nq + 128 * qt:SEQ * kk + QC * nq + 128 * (qt + 1)],
                                rhs=w2_sb[:, HID * kk + 512 * nn:HID * kk + 512 * (nn + 1)],
                                start=(kk == 0), stop=(kk == 1))
                        nc.vector.tensor_tensor(
                            ob[:, 512 * nn:512 * (nn + 1)], pso[:],
                            bob_sb[:, 512 * nn:512 * (nn + 1)],
                            mybir.AluOpType.add)
                    if with_collectives:
                        nc.sync.dma_start(rs_in[128 * qt:128 * (qt + 1), :], ob[:, :HID])
                        if nq == NQ - 1 and qt == 1:
                            do_rs(rs_in[0:256, :], 64, f"{nq}a", 128 * nq)
                    else:
                        nc.sync.dma_start(
                            out_e.ap()[QC * nq + 128 * qt:QC * nq + 128 * (qt + 1), :],
                            ob[:, :HID])
                if with_collectives:
                    if nq == NQ - 1:
                        do_rs(rs_in[256:512, :], 64, f"{nq}b", 128 * nq + 64)
                    else:
                        do_rs(rs_in[:], 128, str(nq), 128 * nq)

            def do_rs(src_ap, rows, label, out_row):
                rs_out = dpool.tile([rows, HID], F32, tag="rsout", bufs=2,
                                    name=f"rsout_{label}")
                nc.gpsimd.collective_compute(
                    "ReduceScatter",
                    mybir.AluOpType.add,
                    replica_groups=[[0, 1, 2, 3], [4, 5, 6, 7]][:max(1, n_cores // 4)],
                    ins=[src_ap.opt()],
                    outs=[rs_out[:].opt()],
                )
                nc.sync.dma_start(out_e.ap()[out_row:out_row + rows, :], rs_out[:])

            # q-chunks: (index, q_off, q_len); the final 512 rows are two
            # 256-row chunks so the drain tail and last RS are half-sized.
            CHUNKS = [(0, 0, 512), (1, 512, 512), (2, 1024, 512),
                      (3, 1536, 256), (4, 1792, 256)]
            # per-chunk output row offset in out_e (rank-relative)
            OUT_ROW = {0: 0, 1: 128, 2: 256, 3: 384, 4: 448}

            # ---- pass 2: per q-chunk attention; out proj of the previous
            # chunk is emitted inside the next chunk's score loop so the PE
            # keeps feeding ACT at chunk boundaries.
            def out_proj(ch):
                idx, q_off, q_len = ch
                nqt = q_len // 128
                rs_in = dpool.tile([QC, HID], F32, tag="rsin", bufs=2,
                                   name=f"rsin_{idx}")
                for qt in range(nqt):
                    ob = wpool.tile([128, HID], F32, tag="ob", bufs=3,
                                    name=f"ob_{idx}_{qt}")
                    for nn in range(2):
                        pso = pp.tile([128, 512], F32, tag="pr", bufs=2,
                                      name=f"pso_{idx}_{qt}_{nn}")
                        for kk in range(2):
                            nc.tensor.matmul(
                                pso[:],
                                lhsT=outT_sb[:, SEQ * kk + q_off + 128 * qt:SEQ * kk + q_off + 128 * (qt + 1)],
                                rhs=w2_sb[:, HID * kk + 512 * nn:HID * kk + 512 * (nn + 1)],
                                start=(kk == 0), stop=(kk == 1))
                        nc.vector.tensor_tensor(
                            ob[:, 512 * nn:512 * (nn + 1)], pso[:],
                            bob_sb[:, 512 * nn:512 * (nn + 1)],
                            mybir.AluOpType.add)
                    if with_collectives:
                        nc.sync.dma_start(rs_in[128 * qt:128 * (qt + 1), :], ob[:, :HID])
                    else:
                        nc.sync.dma_start(
                            out_e.ap()[q_off + 128 * qt:q_off + 128 * (qt + 1), :],
                            ob[:, :HID])
                if with_collectives:
                    do_rs(rs_in[0:q_len, :], q_len // 4, str(idx), OUT_ROW[idx])

            pending = None
            pending_norm = []
            for ch in CHUNKS:
                idx, q_off, q_len = ch
                G = 1024 // q_len
                for hp in range(2):
                    early_A, early_B = (pt_00 if (idx == 0 and hp == 0) else
                                        (pt_01[0], None) if (idx == 0 and hp == 1) else
                                        (None, None))
                    ptA = early_A or wpool.tile([128, KT * q_len], BF16, tag="pt",
                                                bufs=5, name=f"ptA_{idx}_{hp}")
                    ptB = early_B or (pt_01[1] if (idx == 0 and hp == 1) else
                                      wpool.tile([128, KT * q_len], BF16, tag="pt",
                                                 bufs=5, name=f"ptB_{idx}_{hp}"))
                    oaccs = [pp.tile([VW, 512], F32, tag="oacc", bufs=2,
                                     name=f"oacc_{idx}_{2 * hp + half}")
                             for half in range(2)]
                    def v_group(kg):
                        for half, pt in ((0, ptA), (1, ptB)):
                            for j in range(G):
                                kt = G * kg + j
                                v_mm(oaccs[half], 2 * hp + half, kt, pt, q_len,
                                     kt == 0, kt == KT - 1)
                    # V matmuls trail the scores/exp by one group so the PE
                    # never head-of-line blocks on the exp it just requested
                    for kg in range(KT // G):
                        scores_exp(ch, hp, kg,
                                   None if early_A is not None else ptA,
                                   None if early_B is not None else ptB)
                        if kg == 0:
                            for args in pending_norm:
                                normalize(*args)
                            pending_norm = []
                        else:
                            v_group(kg - 1)
                        if kg == 1 and hp == 0 and pending is not None:
                            out_proj(pending)
                            pending = None
                    v_group(KT // G - 1)
                    for half in range(2):
                        pending_norm.append((ch, hp, half, oaccs[half]))
                pending = ch
            for args in pending_norm:
                normalize(*args)
            out_proj(pending)

    nc.compile()
    _NC_CACHE[key] = nc
    return nc


def _prep_in_maps(x, qkv_w, qkv_b, out_w, out_b):
    mats = _rope_mats()
    x = np.asarray(x, np.float32)
    qkv_w = np.asarray(qkv_w, np.float32)
    qkv_b = np.asarray(qkv_b, np.float32)
    out_w = np.asarray(out_w, np.float32)
    out_b = np.asarray(out_b, np.float32)

    # per-head slices of interleaved qkv (head h owns cols 192h .. 192h+192)
    wq = np.stack([qkv_w[:, 192 * h:192 * h + 64] for h in range(HEADS)])      # [16,1024,64]
    wk = np.stack([qkv_w[:, 192 * h + 64:192 * h + 128] for h in range(HEADS)])
    wv = np.stack([qkv_w[:, 192 * h + 128:192 * h + 192] for h in range(HEADS)])
    bq = np.stack([qkv_b[192 * h:192 * h + 64] for h in range(HEADS)])
    bk = np.stack([qkv_b[192 * h + 64:192 * h + 128] for h in range(HEADS)])
    bvv = np.stack([qkv_b[192 * h + 128:192 * h + 192] for h in range(HEADS)])

    import ml_dtypes
    scale = 1.0 / np.sqrt(D)
    wq_r = np.einsum("hij,hjk->hik", wq, mats) * scale
    bq_r = np.einsum("hj,hjk->hk", bq, mats) * scale
    wk_r = np.einsum("hij,hjk->hik", wk, mats)
    bk_r = np.einsum("hj,hjk->hk", bk, mats)

    in_maps = []
    for c in range(N_CORES):
        g, r = divmod(c, 4)
        hs = [4 * r + i for i in range(HPC)]
        xt = _round_tf32(x[g].T)                                            # [1024, 2048]
        wall_c = np.concatenate([wq_r[h] for h in hs] + [wk_r[h] for h in hs]
                                + [wv[h] for h in hs], axis=1)              # [1024, 768]
        w2_c = out_w[256 * r:256 * (r + 1), :]                              # [256, 1024]
        ball_c = np.concatenate([bq_r[h] for h in hs] + [bk_r[h] for h in hs]
                                + [bvv[h] for h in hs])                     # [768]
        bo_c = (out_b[None, :] if r == 0 else np.zeros((1, HID), np.float32))
        in_maps.append({
            "xt": xt,
            "wall": _round_tf32(wall_c),
            "w2": w2_c.astype(ml_dtypes.bfloat16),
            "ball": ball_c.reshape(6, 128).T.copy().astype(np.float32),
            "bo": _round_tf32(bo_c),
            "ones_i": np.ones((1, 128), np.float32),
            "ident": np.eye(128, dtype=ml_dtypes.bfloat16),
            "vones": np.ones((128, KT * HPC), ml_dtypes.bfloat16),
        })
    return in_maps


def kernel(x, qkv_w, qkv_b, out_w, out_b):
    in_maps = _prep_in_maps(x, qkv_w, qkv_b, out_w, out_b)
    nc = _build(with_collectives=True)
    res = None
    for attempt, backoff in enumerate((10, 20, 40, 60, 0)):
        try:
            res = bass_utils.run_bass_kernel_spmd(nc, in_maps,
                                                  core_ids=list(range(N_CORES)))
            break
        except Exception:
            if backoff == 0:
                raise
            import time as _time
            _time.sleep(backoff)
    out = np.empty((2, SEQ, HID), np.float32)
    for c in range(N_CORES):
        g, r = divmod(c, 4)
        o = res.results[c]["out"]            # [512, 1024]
        for j in range(3):
            out[g, 512 * j + 128 * r:512 * j + 128 * r + 128] = o[128 * j:128 * (j + 1)]
        # chunk 3 was reduced as two half-sized ReduceScatters
        for s in range(2):
            out[g, 1536 + 256 * s + 64 * r:1536 + 256 * s + 64 * (r + 1)] = \
                o[384 + 64 * s:384 + 64 * (s + 1)]
    return out
